# revision 1
# baseline (speedup 1.0000x reference)
"""Trainium2 Bass kernel for the CapsuleNetwork routing problem.

Problem (per reference):
  B, L, D, K = 1024, 200, 64, 4 ; E = K*D = 256
  hat[b,l,e] = sum_d seq[b,l,d] * W[l,e,d]          (einsum, PE)
  3 rounds of dynamic routing over interests K (softmax over K per (b,l)),
  cap = squash(w @ hat), cw += hat . cap            (DVE/ACT)
  output cap -> [B, K, D]

Sharding: pure data-parallel over batch across 8 NeuronCores (128 rows each);
weights replicated. All layout transforms (the d'-contraction transposes) are
host-side so the device sees clean burst DMAs.

Restructured routing algebra (validated vs reference to ~3e-7):
  cw layout [B, L, K];   w = exp(cw) / sum_k exp(cw)
  capRaw[b,k,:] = sum_l w[b,l,k] hat[b,l,k,:]
  n = |capRaw|^2 ; s = n/(1+n)/sqrt(n+1e-9)
  cw += s[b,k] * (hat . capRaw)    (squash scale folded into the cw update)
  final out = s * capRaw

Device layout: hat free axis is (d, k) with k innermost so broadcast access
patterns (0-step dims) keep step-1 innermost on every operand, preserving the
DVE 2x bf16 perf mode for the routing streams. Reductions are pairwise
tree-folds (TT adds at 2x for bf16) instead of 1x tensor_reduce.

Streams visit positions parity-major (all even l, then all odd): matmuls
alternating lhsT base_partition 0/64 run in different PE row-groups
CONCURRENTLY and draining two into one PSUM bank fails on hardware;
same-parity ordering keeps each PSUM tile single-row-group, enabling
PSB=4 matmul batching (HW-validated).
"""

import os
import sys

import numpy as np

for _p in ("/opt/trn_rl_repo", "/root/.axon_site/_ro/trn_rl_repo"):
    if os.path.isdir(_p) and _p not in sys.path:
        sys.path.insert(0, _p)

B, L, D, K = 1024, 200, 64, 4
E = K * D
NCORES = 8
BS = B // NCORES  # 128 batch rows per core
M = L // 2        # l-pairs: partition p = (l%2)*64 + d'

# --- tuning flags ---
ROUT_EINSUM_F32R = os.environ.get("KERNEL_F32R", "1") == "1"
PSB = int(os.environ.get("KERNEL_PSB", "4"))  # l's per PSUM tile
PB = 16                   # l's per chunk, routing streams (PSB * n)
PBF = 4                   # l's per chunk, final fp32 stream (PSUM-direct)
PSUM_BUFS = int(os.environ.get("KERNEL_PSUMBUFS", "2"))


def build_nc():
    """Build the Bass program for one core (SPMD; all cores run the same NEFF)."""
    import concourse.bass as bass
    import concourse.tile as tile
    from concourse import bacc, mybir

    f32 = mybir.dt.float32
    f32r = mybir.dt.float32r
    bf16 = mybir.dt.bfloat16
    AF = mybir.ActivationFunctionType
    OP = mybir.AluOpType

    nc = bacc.Bacc(trn_type="TRN2", target_bir_lowering=False, debug=False)
    # seqT/wT typed float32r end-to-end (same bits as fp32 in memory); the
    # routing einsums consume them natively at 1 cyc/row, the final exact
    # pass bitcasts back to float32.
    edt = f32r if ROUT_EINSUM_F32R else f32
    seqT_d = nc.dram_tensor("seqT", [128, M, BS], edt, kind="ExternalInput")
    wT_d = nc.dram_tensor("wT", [128, M, E], edt, kind="ExternalInput")
    cw_d = nc.dram_tensor("cw", [BS, L, K], f32, kind="ExternalInput")
    out_d = nc.dram_tensor("out", [BS, E], f32, kind="ExternalOutput")

    with tile.TileContext(nc) as tc:
        with (
            tc.tile_pool(name="consts", bufs=1) as consts,
            tc.tile_pool(name="hatps", bufs=PSUM_BUFS, space="PSUM") as psum,
            tc.tile_pool(name="hats", bufs=int(os.environ.get("KERNEL_HATBUFS", "2"))) as hats,
            tc.tile_pool(name="scr", bufs=int(os.environ.get("KERNEL_SCRBUFS", "2"))) as scr,
        ):
            seqT = consts.tile([128, M, BS], edt, name="seqT_sb")
            wT = consts.tile([128, M, E], edt, name="wT_sb")
            cw = consts.tile([BS, L, K], f32, name="cw_sb")
            w = consts.tile([BS, L, K], f32, name="w_sb")
            zsum = consts.tile([BS, L], f32, name="zsum")
            zinv = consts.tile([BS, L], f32, name="zinv")
            deltaB = consts.tile([BS, L, K], f32, name="deltaB")
            wB = consts.tile([BS, L, K], bf16, name="wB_sb")
            capB = consts.tile([BS, D, K], bf16, name="capB")
            # capRaw free layout: (d, k) to match hat tiles
            capRaw = consts.tile([BS, D, K], f32, name="capRaw")
            capOut = consts.tile([BS, E], f32, name="capOut")
            # small per-(b,k) scalars packed into one tile (col-sliced)
            smalls = consts.tile([BS, 8, K], f32, name="smalls")
            nvec = smalls[:, 0, :]
            lnt = smalls[:, 1, :]
            rt = smalls[:, 2, :]
            np1 = smalls[:, 3, :]
            den = smalls[:, 4, :]
            dinv = smalls[:, 5, :]
            svec = smalls[:, 6, :]
            epsB = consts.tile([BS, 1], f32, name="epsB")
            nc.vector.memset(epsB[:], 1e-9)

            nc.sync.dma_start(out=cw[:], in_=cw_d[:])
            nc.sync.dma_start(out=seqT[:], in_=seqT_d[:])
            nc.sync.dma_start(out=wT[:], in_=wT_d[:])
            with tc.tile_pool(name="dummyp", bufs=1, space="PSUM") as dummyp:
                dps = dummyp.tile([1, 1], f32, name="dps")
                nc.tensor.matmul(
                    dps[:],
                    lhsT=seqT[0:64, 0, 0:1].bitcast(f32),
                    rhs=seqT[0:64, 0, 0:1].bitcast(f32),
                    start=True,
                    stop=True,
                )

            def softmax():
                # w = softmax over k of cw (k innermost, contiguous)
                nc.scalar.activation(out=w[:], in_=cw[:], func=AF.Exp)
                nc.vector.tensor_reduce(
                    out=zsum[:], in_=w[:], axis=mybir.AxisListType.X, op=OP.add
                )
                nc.vector.reciprocal(out=zinv[:], in_=zsum[:])
                for k in range(K):
                    nc.vector.tensor_mul(out=w[:, :, k], in0=w[:, :, k], in1=zinv[:])
                nc.vector.tensor_copy(out=wB[:], in_=w[:])

            LPB = 512 // E  # l's per PSUM bank (2)

            def emit_one(ps, j, l, use_f32r, grp):
                par, m = l % 2, l // 2
                p0 = 64 * par
                lhsT = seqT[p0 : p0 + 64, m, :]
                rhs = wT[p0 : p0 + 64, m, :]
                if ROUT_EINSUM_F32R and not use_f32r:
                    lhsT = lhsT.bitcast(f32)
                    rhs = rhs.bitcast(f32)
                nc.tensor.matmul(
                    ps[:, j, :], lhsT=lhsT, rhs=rhs,
                    start=(grp % LPB == 0), stop=(grp % LPB == LPB - 1),
                    skip_group_check=True,
                )

            def emit_matmuls(ps, c0, nl, use_f32r):
                for j in range(nl):
                    l = c0 + j
                    par, m = l % 2, l // 2
                    p0 = 64 * par
                    lhsT = seqT[p0 : p0 + 64, m, :]
                    rhs = wT[p0 : p0 + 64, m, :]
                    if ROUT_EINSUM_F32R and not use_f32r:
                        lhsT = lhsT.bitcast(f32)
                        rhs = rhs.bitcast(f32)
                    # start=True clears the whole PSUM bank: only the first
                    # matmul landing in each bank may set it.
                    nc.tensor.matmul(
                        ps[:, j, :],
                        lhsT=lhsT,
                        rhs=rhs,
                        start=(j % LPB == 0),
                        stop=(j % LPB == LPB - 1 or j == nl - 1),
                        skip_group_check=True,
                    )

            HSDT = bf16 if os.environ.get("KERNEL_HS", "bf16") == "bf16" else f32
            NOCONSUME = os.environ.get("KERNEL_NOCONSUME", "0") == "1"

            PARITY = os.environ.get("KERNEL_PARITY", "1") == "1"

            def stream(consume, use_f32r):
                """Routing stream: einsum -> PSUM [128,PSB,E] tiles -> ACT
                copies into one bf16 SBUF chunk -> consume(lbase, lstride,
                hs_bf16, nl). PARITY mode visits all even l then all odd so
                matmuls sharing a PSUM tile share a PE row-group."""
                if PARITY:
                    for half in (0, 1):
                        for mc in range(0, M, PB):
                            nl = min(PB, M - mc)
                            hs = hats.tile([128, PB, E], HSDT, name="hs", tag="hs")
                            for b0 in range(0, nl, PSB):
                                nb = min(PSB, nl - b0)
                                ps = psum.tile(
                                    [128, PSB, E], f32, name="ps", tag="ps"
                                )
                                for j in range(nb):
                                    emit_one(ps, j, 2 * (mc + b0 + j) + half,
                                             use_f32r, j)
                                nc.scalar.copy(
                                    out=hs[:, b0 : b0 + nb, :], in_=ps[:, 0:nb, :]
                                )
                            if not NOCONSUME:
                                consume(2 * mc + half, 2, hs, nl)
                else:
                    for c0 in range(0, L, PB):
                        nl = min(PB, L - c0)
                        hs = hats.tile([128, PB, E], HSDT, name="hs", tag="hs")
                        for b0 in range(0, nl, PSB):
                            ps = psum.tile([128, PSB, E], f32, name="ps", tag="ps")
                            emit_matmuls(ps, c0 + b0, PSB, use_f32r)
                            nc.scalar.copy(out=hs[:, b0 : b0 + PSB, :], in_=ps[:])
                        if not NOCONSUME:
                            consume(c0, 1, hs, nl)

            def capacc_consume(lbase, lstride, hs, nl):
                # u = hs * w-broadcast (bf16 2x), tree-fold l, fp32 add
                u = scr.tile([128, PB, E], bf16, name="u", tag="u")
                win = bass.AP(
                    tensor=wB.tensor,
                    offset=wB.offset + lbase * K,
                    ap=[wB.ap[0], [lstride * K, nl], [0, D], [1, K]],
                )
                nc.vector.tensor_tensor(
                    out=u[:, 0:nl, :], in0=hs[:, 0:nl, :], in1=win, op=OP.mult
                )
                width = nl
                while width > 1:
                    h = width // 2
                    nc.vector.tensor_add(
                        out=u[:, 0:h, :], in0=u[:, 0:h, :], in1=u[:, h : 2 * h, :]
                    )
                    width = h
                nc.vector.tensor_add(out=capRaw[:], in0=capRaw[:], in1=u[:, 0, :])

            def delta_consume(lbase, lstride, hs, nl):
                u = scr.tile([128, PB, E], bf16, name="u", tag="u")
                cin = bass.AP(
                    tensor=capB.tensor,
                    offset=capB.offset,
                    ap=[capB.ap[0], [0, nl], [1, E]],
                )
                nc.vector.tensor_tensor(
                    out=u[:, 0:nl, :], in0=hs[:, 0:nl, :], in1=cin, op=OP.mult
                )
                # fold d (d-major halves of (d,k) are contiguous slabs)
                width = D
                while width > 2:
                    h = width // 2
                    nc.vector.tensor_add(
                        out=u[:, 0:nl, 0 : h * K],
                        in0=u[:, 0:nl, 0 : h * K],
                        in1=u[:, 0:nl, h * K : 2 * h * K],
                    )
                    width = h
                dout = bass.AP(
                    tensor=deltaB.tensor,
                    offset=deltaB.offset + lbase * K,
                    ap=[deltaB.ap[0], [lstride * K, nl], [1, K]],
                )
                nc.vector.tensor_add(
                    out=dout, in0=u[:, 0:nl, 0:K], in1=u[:, 0:nl, K : 2 * K]
                )

            def final_stream():
                """Final cap pass: fp32 einsum, PSUM-direct fp32 mult+folds."""
                for c0 in range(0, L, PBF):
                    ps = psum.tile([128, PBF, E], f32, name="ps", tag="ps")
                    emit_matmuls(ps, c0, PBF, use_f32r=False)
                    u = scr.tile([128, PBF, E], f32, name="uf", tag="u")
                    win = bass.AP(
                        tensor=w.tensor,
                        offset=w.offset + c0 * K,
                        ap=[w.ap[0], [K, PBF], [0, D], [1, K]],
                    )
                    nc.vector.tensor_tensor(out=u[:], in0=ps[:], in1=win, op=OP.mult)
                    width = PBF
                    while width > 1:
                        h = width // 2
                        nc.vector.tensor_add(
                            out=u[:, 0:h, :],
                            in0=u[:, 0:h, :],
                            in1=u[:, h : 2 * h, :],
                        )
                        width = h
                    nc.vector.tensor_add(
                        out=capRaw[:], in0=capRaw[:], in1=u[:, 0, :]
                    )

            def squash_scalars():
                # n[b,k] = sum_d capRaw[b,d,k]^2 via STT accum per k
                for k in range(K):
                    u2 = scr.tile([128, D], f32, name="u2", tag="u2")
                    nc.vector.scalar_tensor_tensor(
                        out=u2[:],
                        in0=capRaw[:, :, k],
                        scalar=1.0,
                        in1=capRaw[:, :, k],
                        op0=OP.mult,
                        op1=OP.mult,
                        accum_out=nvec[:, k : k + 1],
                    )
                # s = n / (1+n) / sqrt(n + 1e-9); sqrt via exp(0.5*ln(x))
                nc.scalar.activation(out=lnt, in_=nvec, func=AF.Ln, bias=epsB[:])
                nc.scalar.activation(out=rt, in_=lnt, func=AF.Exp, scale=0.5)
                nc.vector.tensor_scalar_add(out=np1, in0=nvec, scalar1=1.0)
                nc.vector.tensor_mul(out=den, in0=np1, in1=rt)
                nc.vector.reciprocal(out=dinv, in_=den)
                nc.vector.tensor_mul(out=svec, in0=nvec, in1=dinv)
                nc.vector.tensor_copy(out=capB[:], in_=capRaw[:])

            def cw_update():
                # cw[:, :, k] += s[:, k] * deltaB[:, :, k]
                for k in range(K):
                    nc.vector.scalar_tensor_tensor(
                        out=cw[:, :, k],
                        in0=deltaB[:, :, k],
                        scalar=svec[:, k : k + 1],
                        in1=cw[:, :, k],
                        op0=OP.mult,
                        op1=OP.add,
                    )

            def _lk(t, lbase, lstride, nl, inner=K):
                # [BS, (nl l's at stride lstride), inner] view of an (L,K) tile
                return bass.AP(
                    tensor=t.tensor,
                    offset=t.offset + lbase * K,
                    ap=[t.ap[0], [lstride * K, nl], [1, inner]],
                )

            def _l1(t, lbase, lstride, nl):
                # [BS, nl] view of an (L,) tile at stride lstride
                return bass.AP(
                    tensor=t.tensor,
                    offset=t.offset + lbase,
                    ap=[t.ap[0], [lstride, nl]],
                )

            def softmax_chunk(lbase, lstride, nl):
                nc.scalar.activation(
                    out=_lk(w, lbase, lstride, nl),
                    in_=_lk(cw, lbase, lstride, nl),
                    func=AF.Exp,
                )
                nc.vector.tensor_reduce(
                    out=_l1(zsum, lbase, lstride, nl),
                    in_=_lk(w, lbase, lstride, nl),
                    axis=mybir.AxisListType.X,
                    op=OP.add,
                )
                nc.vector.reciprocal(
                    out=_l1(zinv, lbase, lstride, nl),
                    in_=_l1(zsum, lbase, lstride, nl),
                )
                for k in range(K):
                    nc.vector.tensor_mul(
                        out=_lk(w[:, :, k : k + 1], lbase, lstride, nl, inner=1),
                        in0=_lk(w[:, :, k : k + 1], lbase, lstride, nl, inner=1),
                        in1=_l1(zinv, lbase, lstride, nl),
                    )
                nc.vector.tensor_copy(
                    out=_lk(wB, lbase, lstride, nl),
                    in_=_lk(w, lbase, lstride, nl),
                )

            def fused_consume(lbase, lstride, hs, nl):
                # delta for this chunk, then the per-(b,l)-local routing
                # update (cw += s*delta; softmax), then the next iteration's
                # cap accumulation -- one hat read serves both passes.
                delta_consume(lbase, lstride, hs, nl)
                for k in range(K):
                    cwk = bass.AP(
                        tensor=cw.tensor,
                        offset=cw.offset + lbase * K + k,
                        ap=[cw.ap[0], [lstride * K, nl]],
                    )
                    dbk = bass.AP(
                        tensor=deltaB.tensor,
                        offset=deltaB.offset + lbase * K + k,
                        ap=[deltaB.ap[0], [lstride * K, nl]],
                    )
                    nc.vector.scalar_tensor_tensor(
                        out=cwk,
                        in0=dbk,
                        scalar=svec[:, k : k + 1],
                        in1=cwk,
                        op0=OP.mult,
                        op1=OP.add,
                    )
                softmax_chunk(lbase, lstride, nl)
                capacc_consume(lbase, lstride, hs, nl)

            # ================= routing iterations =================
            if os.environ.get("KERNEL_FUSE", "1") == "1":
                softmax()
                nc.vector.memset(capRaw[:], 0.0)
                stream(capacc_consume, use_f32r=ROUT_EINSUM_F32R)
                squash_scalars()
                for it in (1, 2):
                    nc.vector.memset(capRaw[:], 0.0)
                    stream(fused_consume, use_f32r=ROUT_EINSUM_F32R)
                    squash_scalars()
            else:
                for it in range(3):
                    final = it == 2
                    softmax()
                    nc.vector.memset(capRaw[:], 0.0)
                    stream(capacc_consume, use_f32r=ROUT_EINSUM_F32R)
                    squash_scalars()
                    if not final:
                        stream(delta_consume, use_f32r=ROUT_EINSUM_F32R)
                        cw_update()

            # final: out[b, (k,d)] = s[b,k] * capRaw[b, d, k]  (emit (k,d) order)
            for k in range(K):
                nc.vector.tensor_scalar_mul(
                    out=capOut[:, k * D : (k + 1) * D],
                    in0=capRaw[:, :, k],
                    scalar1=svec[:, k : k + 1],
                )
            nc.sync.dma_start(out=out_d[:], in_=capOut[:])

    nc.finalize()
    return nc


def build_tiny():
    """Minimal kernel (DMA in + copy + DMA out) for dispatch-overhead baseline."""
    import concourse.tile as tile
    from concourse import bacc, mybir

    f32 = mybir.dt.float32
    nc = bacc.Bacc(trn_type="TRN2", target_bir_lowering=False, debug=False)
    cw_d = nc.dram_tensor("cw", [BS, L, K], f32, kind="ExternalInput")
    out_d = nc.dram_tensor("out", [BS, E], f32, kind="ExternalOutput")
    with tile.TileContext(nc) as tc:
        with tc.tile_pool(name="p", bufs=1) as p:
            t = p.tile([BS, L, K], f32, name="t_sb")
            o = p.tile([BS, E], f32, name="o_sb")
            nc.sync.dma_start(out=t[:], in_=cw_d[:])
            nc.vector.tensor_copy(out=o[:], in_=t[:, 0:64, :])
            nc.sync.dma_start(out=out_d[:], in_=o[:])
    nc.finalize()
    return nc


_NC_CACHE = None


def _get_nc():
    global _NC_CACHE
    if _NC_CACHE is None:
        _NC_CACHE = build_nc()
    return _NC_CACHE


def prep_inputs(seq_out, weights, capsule_weight):
    """Host-side layout prep -> list of per-core input maps."""
    seq = np.ascontiguousarray(np.asarray(seq_out, dtype=np.float32))
    W = np.ascontiguousarray(np.asarray(weights, dtype=np.float32))[0]  # [L,E,D]
    cwf = np.ascontiguousarray(np.asarray(capsule_weight, dtype=np.float32))

    # seqT[p, m, b] = seq[b, 2m + p//64, p%64]
    seqT = np.ascontiguousarray(
        seq.reshape(B, M, 2, D).transpose(2, 3, 1, 0).reshape(128, M, B)
    )
    # wT[p, m, (d,k)] = W[2m + p//64, k*D + d, p%64]   (hat free axis = (d,k))
    wTf = W.reshape(M, 2, K, D, D).transpose(1, 4, 0, 3, 2)  # [par, d', m, d, k]
    wT = np.ascontiguousarray(wTf.reshape(128, M, E))
    # cwA[b, l, k] = cw[b, k, l]
    cwA = np.ascontiguousarray(cwf.transpose(0, 2, 1))  # [B, L, K]

    in_maps = []
    for c in range(NCORES):
        in_maps.append(
            {
                "seqT": np.ascontiguousarray(seqT[:, :, c * BS : (c + 1) * BS]),
                "wT": wT,
                "cw": np.ascontiguousarray(cwA[c * BS : (c + 1) * BS]),
            }
        )
    return in_maps


def gather_out(results):
    """Per-core 'out' [BS, E=(k*D+d)] -> full [B, K, D]."""
    return np.concatenate(
        [r["out"].reshape(BS, K, D) for r in results], axis=0
    ).astype(np.float32)


def kernel(seq_out, mask, weights, capsule_weight):
    from concourse.bass_utils import run_bass_kernel_spmd

    nc = _get_nc()
    in_maps = prep_inputs(seq_out, weights, capsule_weight)
    res = run_bass_kernel_spmd(nc, in_maps, core_ids=list(range(NCORES)))
    return gather_out(res.results)


if __name__ == "__main__":
    rng = np.random.default_rng(0)
    seq_out = rng.standard_normal((B, L, D), dtype=np.float32)
    mask = np.ones((B, L), dtype=np.float32)
    weights = (0.02 * rng.standard_normal((1, L, E, D))).astype(np.float32)
    capsule_weight = rng.standard_normal((B, K, L), dtype=np.float32)
    out = kernel(seq_out, mask, weights, capsule_weight)
    print("out", out.shape, out.dtype, float(np.abs(out).max()))



# revision 3
# speedup vs baseline: 1.9922x; 1.9922x over previous
"""Trainium2 Bass kernel for the CapsuleNetwork routing problem (v2).

Problem (per reference):
  B, L, D, K = 1024, 200, 64, 4 ; E = K*D = 256
  hat[b,l,e] = sum_d seq[b,l,d] * W[l,e,d]          (einsum, PE)
  3 rounds of dynamic routing over interests K (softmax over K per (b,l)),
  cap = squash(w @ hat), cw += hat . cap
  output cap -> [B, K, D]

Sharding: pure data-parallel over batch across 8 NeuronCores (BS=128 rows
each); weights replicated. Host-side layout prep gives the device clean
burst DMAs and parity-major (par, m) slot order, l = 2m + par.

v2 architecture (vs the v1 3x-einsum-recompute kernel):
  * All inputs bf16 (halves DMA bytes); DMAs issued from SP + ACT + GPSIMD
    queues concurrently (transfers overlap across issuing engines).
  * hat computed ONCE (bf16, SBUF-resident, 100KiB/partition) -- einsum and
    the PSUM->SBUF copies happen once instead of 3x.
  * The over-l capacc reduction runs on the otherwise-idle PE as
    identity-matmul PSUM accumulation (fp32, better precision than bf16
    trees), freeing the DVE.
  * Remaining elementwise work (routing mults + delta d-folds) is split
    DVE/GPSIMD by per-chunk schedule tables (GPSIMD ~1.5x DVE cost/elem in
    the cost model but otherwise idle).
  * cw update + softmax run bulk per iteration (few big ops), not per chunk.

Restructured routing algebra (validated in v1 vs reference to ~3e-7):
  cw layout [B, slot, K];  w = exp(cw) / sum_k exp(cw)
  capRaw[b,(d,k)] = sum_slot w[b,slot,k] hat[b,slot,(d,k)]
  n = |capRaw|^2 ; s = n/(1+n)/sqrt(n+1e-9)
  cw += s[b,k] * (hat . capRaw)   (squash scale folded into the cw update)
  final out[b,(k,d)] = s * capRaw
"""

import os
import sys

import numpy as np

for _p in ("/opt/trn_rl_repo", "/root/.axon_site/_ro/trn_rl_repo"):
    if os.path.isdir(_p) and _p not in sys.path:
        sys.path.insert(0, _p)

B, L, D, K = 1024, 200, 64, 4
E = K * D
NCORES = 8
BS = B // NCORES  # 128 batch rows per core
M = L // 2        # l = 2m + par ; slot = par*M + m

MCW = 20   # m's per wT DMA chunk (5 chunks)
PSB = 4    # m's per einsum PSUM tile (single parity)
NL = 16    # slots per routing chunk -> 13 chunks (12x16 + 8)

NCHUNK = (L + NL - 1) // NL


def _sched(env, default):
    s = os.environ.get(env, default)
    out = s.split(",")
    assert len(out) == NCHUNK, f"{env}: need {NCHUNK} entries, got {len(out)}"
    return out


# Engine schedules: D=DVE, G=GPSIMD, P=PE(identity-matmul fold), A=ACT.
A_MULT = _sched("KERNEL_A_MULT", "D,D,D,D,D,D,D,D,D,D,D,D,D")
A_FOLD = _sched("KERNEL_A_FOLD", "P,P,G,P,G,P,G,P,G,D,P,D,P")
B_DM = _sched("KERNEL_B_DM", "D,D,G,D,D,G,D,D,G,D,D,G,G")
B_DF = _sched("KERNEL_B_DF", "D,D,G,D,D,G,D,D,G,D,D,G,G")
B_CM = _sched("KERNEL_B_CM", "D,D,G,D,D,G,D,D,G,D,D,G,G")
# einsum PSUM->SBUF hat copy engine: every COPY_GPS_MOD'th copy on GPSIMD
COPY_GPS_MOD = int(os.environ.get("KERNEL_COPY_GPS_MOD", "3"))


def _chunk_slots(c):
    s0 = NL * c
    return s0, min(NL, L - s0)


def _chunk_ready_ci(c):
    """First wT-chunk index ci after which einsum has produced all hat slots
    of routing chunk c (einsum emits both parities per ci block)."""
    s0, nl = _chunk_slots(c)
    need = 0
    for s in range(s0, s0 + nl):
        m = s % M
        need = max(need, m // MCW)
    return need


def build_nc():
    """Build the Bass program for one core (SPMD; all cores run same NEFF)."""
    import concourse.bass as bass
    import concourse.tile as tile
    from concourse import bacc, mybir

    f32 = mybir.dt.float32
    bf16 = mybir.dt.bfloat16
    AF = mybir.ActivationFunctionType
    OP = mybir.AluOpType

    nc = bacc.Bacc(trn_type="TRN2", target_bir_lowering=False, debug=False)
    seqT_d = nc.dram_tensor("seqT", [128, M, BS], bf16, kind="ExternalInput")
    wT_d = nc.dram_tensor("wT", [128, M, E], bf16, kind="ExternalInput")
    cw_d = nc.dram_tensor("cw", [BS, L, K], f32, kind="ExternalInput")
    ident_d = nc.dram_tensor("ident", [128, 128], bf16, kind="ExternalInput")
    out_d = nc.dram_tensor("out", [BS, E], f32, kind="ExternalOutput")

    ENG = None  # set inside context

    with tile.TileContext(nc) as tc:
        with (
            tc.tile_pool(name="consts", bufs=1) as consts,
            tc.tile_pool(name="wtp", bufs=2) as wtp,
            tc.tile_pool(name="scrd", bufs=int(os.environ.get("KERNEL_SCRD", "3"))) as scrd,
            tc.tile_pool(name="scrg", bufs=int(os.environ.get("KERNEL_SCRG", "2"))) as scrg,
            tc.tile_pool(name="pse", bufs=2, space="PSUM") as pse,
            tc.tile_pool(name="psc", bufs=2, space="PSUM") as pscp,
        ):
            seqT = consts.tile([128, M, BS], bf16, name="seqT_sb")
            ident = consts.tile([128, 128], bf16, name="ident_sb")
            hat = consts.tile([BS, L, E], bf16, name="hat_sb")
            cw = consts.tile([BS, L, K], f32, name="cw_sb")
            w = consts.tile([BS, L, K], f32, name="w_sb")
            wB = consts.tile([BS, L, K], bf16, name="wB_sb")
            zsum = consts.tile([BS, L], f32, name="zsum")
            zinv = consts.tile([BS, L], f32, name="zinv")
            deltaB = consts.tile([BS, L, K], f32, name="deltaB")
            capB = consts.tile([BS, E], bf16, name="capB")
            capRaw = consts.tile([BS, D, K], f32, name="capRaw")
            capAccD = consts.tile([BS, D, K], f32, name="capAccD")
            capAccG = consts.tile([BS, D, K], f32, name="capAccG")
            capOut = consts.tile([BS, E], f32, name="capOut")
            smalls = consts.tile([BS, 8, K], f32, name="smalls")
            nvec = smalls[:, 0, :]
            lnt = smalls[:, 1, :]
            rt = smalls[:, 2, :]
            np1 = smalls[:, 3, :]
            den = smalls[:, 4, :]
            dinv = smalls[:, 5, :]
            svec = smalls[:, 6, :]
            epsB = consts.tile([BS, 1], f32, name="epsB")
            u2s = consts.tile([BS, D], f32, name="u2s")
            nc.vector.memset(epsB[:], 1e-9)

            ENG = {"D": nc.vector, "G": nc.gpsimd}

            # ---------------- DMAs (parallel issue queues) ----------------
            nc.scalar.dma_start(out=cw[:], in_=cw_d[:])
            nc.sync.dma_start(out=ident[:], in_=ident_d[:])
            H = M // 2
            nc.gpsimd.dma_start(out=seqT[:, 0:H, :], in_=seqT_d[:, 0:H, :])
            nc.scalar.dma_start(out=seqT[:, H:M, :], in_=seqT_d[:, H:M, :])

            # ---------------- helpers ----------------
            def wb_bcast(s0, nl):
                return bass.AP(
                    tensor=wB.tensor,
                    offset=wB.offset + s0 * K,
                    ap=[wB.ap[0], [K, nl], [0, D], [1, K]],
                )

            def capb_bcast(nl):
                return bass.AP(
                    tensor=capB.tensor,
                    offset=capB.offset,
                    ap=[capB.ap[0], [0, nl], [1, E]],
                )

            def zinv_bcast():
                return bass.AP(
                    tensor=zinv.tensor,
                    offset=zinv.offset,
                    ap=[zinv.ap[0], [1, L], [0, K]],
                )

            def softmax_bulk():
                nc.scalar.activation(out=w[:], in_=cw[:], func=AF.Exp)
                nc.vector.tensor_reduce(
                    out=zsum[:], in_=w[:], axis=mybir.AxisListType.X, op=OP.add
                )
                nc.vector.reciprocal(out=zinv[:], in_=zsum[:])
                nc.vector.tensor_tensor(
                    out=wB[:], in0=w[:], in1=zinv_bcast(), op=OP.mult
                )

            def squash():
                # n[b,k] = sum_d capRaw[b,d,k]^2 ; s = n/(1+n)/sqrt(n+1e-9)
                for k in range(K):
                    nc.vector.scalar_tensor_tensor(
                        out=u2s[:],
                        in0=capRaw[:, :, k],
                        scalar=1.0,
                        in1=capRaw[:, :, k],
                        op0=OP.mult,
                        op1=OP.mult,
                        accum_out=nvec[:, k : k + 1],
                    )
                nc.scalar.activation(out=lnt, in_=nvec, func=AF.Ln, bias=epsB[:])
                nc.scalar.activation(out=rt, in_=lnt, func=AF.Exp, scale=0.5)
                nc.vector.tensor_scalar_add(out=np1, in0=nvec, scalar1=1.0)
                nc.vector.tensor_mul(out=den, in0=np1, in1=rt)
                nc.vector.reciprocal(out=dinv, in_=den)
                nc.vector.tensor_mul(out=svec, in0=nvec, in1=dinv)
                nc.vector.tensor_copy(out=capB[:], in_=capRaw[:])

            def fold_tree_l(eng, u, nl, acc):
                """Sum u[:, 0:nl, :] over slots (nl power of 2), add into acc."""
                width = nl
                while width > 1:
                    h = width // 2
                    eng.tensor_tensor(
                        out=u[:, 0:h, :],
                        in0=u[:, 0:h, :],
                        in1=u[:, h : 2 * h, :],
                        op=OP.add,
                    )
                    width = h
                eng.tensor_tensor(
                    out=acc[:], in0=acc[:], in1=u[:, 0, :], op=OP.add
                )

            def fold_tree_d(eng, u, nl, s0):
                """delta[:, slot, k] = sum_d u[:, slot, (d,k)] -> deltaB."""
                width = D
                while width > 2:
                    h = width // 2
                    eng.tensor_tensor(
                        out=u[:, 0:nl, 0 : h * K],
                        in0=u[:, 0:nl, 0 : h * K],
                        in1=u[:, 0:nl, h * K : 2 * h * K],
                        op=OP.add,
                    )
                    width = h
                eng.tensor_tensor(
                    out=deltaB[:, s0 : s0 + nl, :],
                    in0=u[:, 0:nl, 0:K],
                    in1=u[:, 0:nl, K : 2 * K],
                    op=OP.add,
                )

            def utile(eng_key, tag):
                pool = scrd if eng_key == "D" else scrg
                return pool.tile([BS, NL, E], bf16, name=f"u{eng_key}", tag=f"u{eng_key}")

            # initial softmax (needs only cw)
            softmax_bulk()
            nc.vector.memset(capAccD[:], 0.0)
            nc.vector.memset(capAccG[:], 0.0)

            # ---------------- Phase A: einsum + hat + capacc_0 ----------------
            # chunk emission order by readiness
            order = sorted(range(NCHUNK), key=lambda c: (_chunk_ready_ci(c), c))
            pe_chunks = [c for c in order if A_FOLD[c] == "P"]
            psc0 = pscp.tile([128, 512], f32, name="psc", tag="psc")

            copy_idx = 0
            emitted = 0

            def emit_capacc0(c):
                nonlocal copy_idx
                s0, nl = _chunk_slots(c)
                me = A_MULT[c]
                u = utile(me, "a")
                ENG[me].tensor_tensor(
                    out=u[:, 0:nl, :],
                    in0=hat[:, s0 : s0 + nl, :],
                    in1=wb_bcast(s0, nl),
                    op=OP.mult,
                )
                fm = A_FOLD[c]
                if fm == "P":
                    first = c == pe_chunks[0]
                    last = c == pe_chunks[-1]
                    for j in range(nl):
                        nc.tensor.matmul(
                            psc0[:, 0:E],
                            lhsT=ident[:],
                            rhs=u[:, j, :],
                            start=(first and j == 0),
                            stop=(last and j == nl - 1),
                            skip_group_check=True,
                        )
                elif fm == "D":
                    fold_tree_l(nc.vector, u, nl, capAccD)
                else:
                    fold_tree_l(nc.gpsimd, u, nl, capAccG)

            for ci in range(M // MCW):
                m0 = ci * MCW
                wtc = wtp.tile([128, MCW, E], bf16, name="wtc", tag="wtc")
                nc.sync.dma_start(out=wtc[:], in_=wT_d[:, m0 : m0 + MCW, :])
                for par in (0, 1):
                    p0 = 64 * par
                    for g in range(MCW // PSB):
                        ps = pse.tile([128, PSB, E], f32, name="pse", tag="pse")
                        for j in range(PSB):
                            mo = g * PSB + j
                            nc.tensor.matmul(
                                ps[:, j, :],
                                lhsT=seqT[p0 : p0 + 64, m0 + mo, :],
                                rhs=wtc[p0 : p0 + 64, mo, :],
                                start=(j % 2 == 0),
                                stop=(j % 2 == 1),
                                skip_group_check=True,
                            )
                        dst = hat[:, par * M + m0 + g * PSB : par * M + m0 + g * PSB + PSB, :]
                        if copy_idx % COPY_GPS_MOD == COPY_GPS_MOD - 1:
                            nc.gpsimd.tensor_copy(out=dst, in_=ps[:])
                        else:
                            nc.scalar.copy(out=dst, in_=ps[:])
                        copy_idx += 1
                # emit routing chunks that are now fully covered
                while emitted < NCHUNK and _chunk_ready_ci(order[emitted]) <= ci:
                    emit_capacc0(order[emitted])
                    emitted += 1

            # capRaw = psc0 + tree partials
            nc.scalar.copy(out=capRaw[:], in_=psc0[:, 0:E])
            if "D" in A_FOLD:
                nc.vector.tensor_tensor(
                    out=capRaw[:], in0=capRaw[:], in1=capAccD[:], op=OP.add
                )
            if "G" in A_FOLD:
                nc.vector.tensor_tensor(
                    out=capRaw[:], in0=capRaw[:], in1=capAccG[:], op=OP.add
                )
            squash()

            # ---------------- Phase B: iterations 1, 2 ----------------
            for it in (1, 2):
                # delta pass: deltaB[b,slot,k] = sum_d hat*capB
                for c in range(NCHUNK):
                    s0, nl = _chunk_slots(c)
                    me = B_DM[c]
                    u = utile(me, "b")
                    ENG[me].tensor_tensor(
                        out=u[:, 0:nl, :],
                        in0=hat[:, s0 : s0 + nl, :],
                        in1=capb_bcast(nl),
                        op=OP.mult,
                    )
                    fold_tree_d(ENG[B_DF[c]], u, nl, s0)
                # bulk cw update + softmax
                for k in range(K):
                    nc.vector.scalar_tensor_tensor(
                        out=cw[:, :, k],
                        in0=deltaB[:, :, k],
                        scalar=svec[:, k : k + 1],
                        in1=cw[:, :, k],
                        op0=OP.mult,
                        op1=OP.add,
                    )
                softmax_bulk()
                # capacc pass: capRaw = sum_slot w*hat via PE identity-matmuls
                psc = pscp.tile([128, 512], f32, name="psc", tag="psc")
                for c in range(NCHUNK):
                    s0, nl = _chunk_slots(c)
                    me = B_CM[c]
                    u = utile(me, "b")
                    ENG[me].tensor_tensor(
                        out=u[:, 0:nl, :],
                        in0=hat[:, s0 : s0 + nl, :],
                        in1=wb_bcast(s0, nl),
                        op=OP.mult,
                    )
                    for j in range(nl):
                        nc.tensor.matmul(
                            psc[:, 0:E],
                            lhsT=ident[:],
                            rhs=u[:, j, :],
                            start=(c == 0 and j == 0),
                            stop=(c == NCHUNK - 1 and j == nl - 1),
                            skip_group_check=True,
                        )
                nc.scalar.copy(out=capRaw[:], in_=psc[:, 0:E])
                squash()

            # final: out[b, (k,d)] = s[b,k] * capRaw[b, d, k]
            for k in range(K):
                nc.vector.tensor_scalar_mul(
                    out=capOut[:, k * D : (k + 1) * D],
                    in0=capRaw[:, :, k],
                    scalar1=svec[:, k : k + 1],
                )
            nc.sync.dma_start(out=out_d[:], in_=capOut[:])

    nc.finalize()
    return nc


_NC_CACHE = None


def _get_nc():
    global _NC_CACHE
    if _NC_CACHE is None:
        _NC_CACHE = build_nc()
    return _NC_CACHE


def prep_inputs(seq_out, weights, capsule_weight):
    """Host-side layout prep -> list of per-core input maps."""
    import ml_dtypes

    bf16 = ml_dtypes.bfloat16
    seq = np.ascontiguousarray(np.asarray(seq_out, dtype=np.float32))
    W = np.ascontiguousarray(np.asarray(weights, dtype=np.float32))[0]  # [L,E,D]
    cwf = np.ascontiguousarray(np.asarray(capsule_weight, dtype=np.float32))

    # seqT[p=(64*par+d'), m, b] = seq[b, 2m+par, d']
    seqT = np.ascontiguousarray(
        seq.reshape(B, M, 2, D).transpose(2, 3, 1, 0).reshape(128, M, B).astype(bf16)
    )
    # wT[p, m, (d*K+k)] = W[2m+par, k*D+d, d']
    wTf = W.reshape(M, 2, K, D, D).transpose(1, 4, 0, 3, 2)  # [par, d', m, d, k]
    wT = np.ascontiguousarray(wTf.reshape(128, M, E).astype(bf16))
    # cwA[b, slot=(par*M+m), k] = cw[b, k, 2m+par]
    cwA = np.ascontiguousarray(
        cwf.reshape(B, K, M, 2).transpose(0, 3, 2, 1).reshape(B, L, K)
    )
    ident = np.eye(128, dtype=bf16)

    in_maps = []
    for c in range(NCORES):
        in_maps.append(
            {
                "seqT": np.ascontiguousarray(seqT[:, :, c * BS : (c + 1) * BS]),
                "wT": wT,
                "cw": np.ascontiguousarray(cwA[c * BS : (c + 1) * BS]),
                "ident": ident,
            }
        )
    return in_maps


def gather_out(results):
    """Per-core 'out' [BS, E=(k*D+d)] -> full [B, K, D]."""
    return np.concatenate(
        [np.asarray(r["out"]).reshape(BS, K, D) for r in results], axis=0
    ).astype(np.float32)


def kernel(seq_out, mask, weights, capsule_weight):
    from concourse.bass_utils import run_bass_kernel_spmd

    nc = _get_nc()
    in_maps = prep_inputs(seq_out, weights, capsule_weight)
    res = run_bass_kernel_spmd(nc, in_maps, core_ids=list(range(NCORES)))
    return gather_out(res.results)


if __name__ == "__main__":
    rng = np.random.default_rng(0)
    seq_out = rng.standard_normal((B, L, D), dtype=np.float32)
    mask = np.ones((B, L), dtype=np.float32)
    weights = (0.02 * rng.standard_normal((1, L, E, D))).astype(np.float32)
    capsule_weight = rng.standard_normal((B, K, L)).astype(np.float32)
    out = kernel(seq_out, mask, weights, capsule_weight)
    print("out", out.shape, out.dtype, float(np.abs(out).max()))
